# revision 16
# baseline (speedup 1.0000x reference)
"""Local (sparse) attention layer on 8 Trainium2 NeuronCores.

Sharding: core c handles batch b = c//2, query half c%2 (1024 queries),
full context of its batch (data parallel on the small Dense weights).

v4 pipeline (per core), all heavy data in bf16:
  Host prep: x^T, ctx^T, weights pre-cast to bf16 (Wq/bq pre-scaled by
  1/sqrt(hd)); neighbor indices pre-wrapped into the dma_gather int16
  channel layout.
  A. PE projections straight from the host-transposed activations:
     q (n-major, SBUF), packed [k|v] bf16 rows -> HBM.
  B. Per 128-query tile, per 16-neighbor half-gather j (software
     pipelined: score phase of j overlaps AV phase of j-1):
       score: dma_gather 2x1024 kv rows; DVE dot-products via in-place
         multiply + bf16 pairwise tree over head_dim (TensorReduce runs
         at 1 elem/cycle, the tree runs at 2x); Act exponentiates with
         broadcast over head_dim into an expanded weight tile.
       AV: DVE weights V rows in-place and tree-reduces over neighbors
         (DVE for the big rounds, Pool for the small ones), f32
         accumulation across the two halves (exact flash accumulation:
         scores are O(1), no max shift needed).
     Normalize, PE out-projection, DMA out.
"""

import numpy as np

HEADS = 8
HD = 64
DIM = 512
DIN = 256
B, N, M, K = 4, 2048, 2048, 32
N_LOC = 1024  # queries per core
NT = N_LOC // 128  # query tiles per core
KH = 16  # neighbors per half-gather
NH = K // KH  # half-gathers per tile (2)
NJ = NT * NH  # pipelined half-gather stages

_CACHE = {}


def _build():
    import concourse.bass as bass
    import concourse.bacc as bacc
    import concourse.mybir as mybir
    from concourse.tile import TileContext
    from concourse.masks import make_identity

    f32 = mybir.dt.float32
    bf16 = mybir.dt.bfloat16
    i16 = mybir.dt.int16

    nc = bacc.Bacc("TRN2")
    xT_h = nc.dram_tensor("xT", [DIN, N_LOC], bf16, kind="ExternalInput")
    cT_h = nc.dram_tensor("cT", [DIN, M], bf16, kind="ExternalInput")
    idx_h = nc.dram_tensor("idx", [128, NJ * 128], i16, kind="ExternalInput")
    wq_h = nc.dram_tensor("wq", [DIN, DIM], bf16, kind="ExternalInput")
    wk_h = nc.dram_tensor("wk", [DIN, DIM], bf16, kind="ExternalInput")
    wv_h = nc.dram_tensor("wv", [DIN, DIM], bf16, kind="ExternalInput")
    wo_h = nc.dram_tensor("wo", [DIM, DIN], bf16, kind="ExternalInput")
    bq_h = nc.dram_tensor("bq", [128, DIM], f32, kind="ExternalInput")
    bo_h = nc.dram_tensor("bo", [128, DIN], f32, kind="ExternalInput")
    out_h = nc.dram_tensor("out", [N_LOC, DIN], f32, kind="ExternalOutput")
    kv_h = nc.dram_tensor("kv_scratch", [M, 2 * DIM], bf16, kind="Internal")

    with TileContext(nc) as tc:
        with tc.tile_pool(name="const", bufs=1) as cpool:
            ident = cpool.tile([128, 128], bf16)
            make_identity(nc, ident[:])
            wq_sb = [cpool.tile([128, DIM], bf16, tag=f"wq{c}", name=f"wq{c}") for c in range(2)]
            wk_sb = [cpool.tile([128, DIM], bf16, tag=f"wk{c}", name=f"wk{c}") for c in range(2)]
            wv_sb = [cpool.tile([128, DIM], bf16, tag=f"wv{c}", name=f"wv{c}") for c in range(2)]
            wo_sb = [cpool.tile([128, DIN], bf16, tag=f"wo{c}", name=f"wo{c}") for c in range(4)]
            bq_sb = cpool.tile([128, DIM], f32)
            bo_sb = cpool.tile([128, DIN], f32)
            idx_sb = cpool.tile([128, NJ * 128], i16)
            for c in range(2):
                nc.sync.dma_start(out=wq_sb[c][:], in_=wq_h[c * 128:(c + 1) * 128, :])
                nc.sync.dma_start(out=wk_sb[c][:], in_=wk_h[c * 128:(c + 1) * 128, :])
                nc.sync.dma_start(out=wv_sb[c][:], in_=wv_h[c * 128:(c + 1) * 128, :])
            for c in range(4):
                nc.sync.dma_start(out=wo_sb[c][:], in_=wo_h[c * 128:(c + 1) * 128, :])
            nc.sync.dma_start(out=bq_sb[:], in_=bq_h[:])
            nc.sync.dma_start(out=bo_sb[:], in_=bo_h[:])
            nc.sync.dma_start(out=idx_sb[:], in_=idx_h[:])

            with tc.tile_pool(name="qpool", bufs=1) as qpool:
                q_sb = [qpool.tile([128, DIM], bf16, tag=f"q{t}", name=f"q{t}") for t in range(NT)]

                # ---- phase A: projections ----
                with (
                    tc.tile_pool(name="inp", bufs=1) as ipool,
                    tc.tile_pool(name="stage", bufs=4) as stpool,
                    tc.tile_pool(name="psA", bufs=2, space="PSUM") as psA,
                ):
                    xT_sb = [ipool.tile([128, N_LOC], bf16, tag=f"xT{c}", name=f"xT{c}") for c in range(2)]
                    cT_sb = [ipool.tile([128, M], bf16, tag=f"cT{c}", name=f"cT{c}") for c in range(2)]
                    for c in range(2):
                        nc.sync.dma_start(out=xT_sb[c][:], in_=xT_h[c * 128:(c + 1) * 128, :])
                        nc.sync.dma_start(out=cT_sb[c][:], in_=cT_h[c * 128:(c + 1) * 128, :])
                    for mt in range(M // 128):
                        psk = psA.tile([128, DIM], f32, tag="mmk")
                        psv = psA.tile([128, DIM], f32, tag="mmv")
                        for c in range(2):
                            nc.tensor.matmul(
                                out=psk[:], lhsT=cT_sb[c][:, mt * 128:(mt + 1) * 128],
                                rhs=wk_sb[c][:], start=(c == 0), stop=(c == 1))
                        for c in range(2):
                            nc.tensor.matmul(
                                out=psv[:], lhsT=cT_sb[c][:, mt * 128:(mt + 1) * 128],
                                rhs=wv_sb[c][:], start=(c == 0), stop=(c == 1))
                        kvt = stpool.tile([128, 2 * DIM], bf16, tag="kvt")
                        if mt % 2 == 0:
                            nc.scalar.activation(
                                out=kvt[:, :DIM], in_=psk[:],
                                func=mybir.ActivationFunctionType.Copy)
                            nc.vector.tensor_copy(out=kvt[:, DIM:], in_=psv[:])
                        else:
                            nc.vector.tensor_copy(out=kvt[:, :DIM], in_=psk[:])
                            nc.scalar.activation(
                                out=kvt[:, DIM:], in_=psv[:],
                                func=mybir.ActivationFunctionType.Copy)
                        nc.sync.dma_start(
                            out=kv_h[mt * 128:(mt + 1) * 128, :], in_=kvt[:])
                    for t in range(NT):
                        psq = psA.tile([128, DIM], f32, tag="mmq")
                        for c in range(2):
                            nc.tensor.matmul(
                                out=psq[:], lhsT=xT_sb[c][:, t * 128:(t + 1) * 128],
                                rhs=wq_sb[c][:], start=(c == 0), stop=(c == 1))
                        nc.vector.tensor_tensor(
                            out=q_sb[t][:], in0=psq[:], in1=bq_sb[:],
                            op=mybir.AluOpType.add)

                # ---- phase B: software-pipelined gather + attention ----
                with (
                    tc.tile_pool(name="gat", bufs=3) as gpool,
                    tc.tile_pool(name="sco", bufs=3) as spool,
                    tc.tile_pool(name="eexp", bufs=3) as epool,
                    tc.tile_pool(name="red", bufs=3) as rpool,
                    tc.tile_pool(name="acc", bufs=3) as apool,
                    tc.tile_pool(name="psT", bufs=2, space="PSUM") as psT,
                    tc.tile_pool(name="psO", bufs=2, space="PSUM") as psO,
                ):
                    kvgs, eexs, sts = {}, {}, {}
                    avs, dens = {}, {}

                    def score_phase(j):
                        t = j // NH
                        kvg = gpool.tile([128, KH, 2 * DIM], bf16, tag="kvg", name="kvg")
                        kvgs[j] = kvg
                        for g in range(2):
                            col0 = j * 128 + g * 64
                            nc.gpsimd.dma_gather(
                                out_ap=kvg[:, g * (KH // 2):(g + 1) * (KH // 2), :],
                                in_ap=kv_h[:],
                                idxs_ap=idx_sb[:, col0:col0 + 64],
                                num_idxs=KH * 64,
                                num_idxs_reg=KH * 64,
                                elem_size=2 * DIM,
                            )
                        kg = kvg[:, :, :DIM].rearrange(
                            "p k (h d) -> p k h d", h=HEADS)
                        # scores in-place into the gathered K half
                        nc.vector.tensor_tensor(
                            out=kg, in0=kg,
                            in1=q_sb[t][:].rearrange(
                                "p (o h d) -> p o h d", o=1, h=HEADS
                            ).to_broadcast([128, KH, HEADS, HD]),
                            op=mybir.AluOpType.mult)
                        # bf16 pairwise tree over head_dim (2x DVE rate),
                        # then a small f32 reduce over the last 4
                        with nc.allow_low_precision(reason="bf16 dot tree, f32 finish"):
                            for eng, w in ((nc.vector, 32), (nc.vector, 16), (nc.vector, 8), (nc.vector, 4)):
                                eng.tensor_tensor(
                                    out=kg[:, :, :, :w], in0=kg[:, :, :, :w],
                                    in1=kg[:, :, :, w:2 * w],
                                    op=mybir.AluOpType.add)

                    def score_back(j):
                        kvg = kvgs[j]
                        kg = kvg[:, :, :DIM].rearrange(
                            "p k (h d) -> p k h d", h=HEADS)
                        s = spool.tile([128, KH, HEADS], f32, tag="s", name="s")
                        sts[j] = s
                        nc.vector.tensor_reduce(
                            out=s[:], in_=kg[:, :, :, :4],
                            axis=mybir.AxisListType.X,
                            op=mybir.AluOpType.add)
                        # exp(s) broadcast-expanded over head_dim (Act)
                        eex = epool.tile([128, KH, HEADS, HD], bf16, tag="eex", name="eex")
                        eexs[j] = eex
                        nc.scalar.activation(
                            out=eex[:],
                            in_=s[:].rearrange(
                                "p k (h o) -> p k h o", o=1
                            ).to_broadcast([128, KH, HEADS, HD]),
                            func=mybir.ActivationFunctionType.Exp)

                    def av_phase(j):
                        t, h2 = j // NH, j % NH
                        kvg, eex = kvgs.pop(j), eexs.pop(j)
                        if h2 == 0:
                            avs[t] = apool.tile([128, DIM], f32, tag="av", name="av")
                            dens[t] = apool.tile([128, HEADS], f32, tag="den", name="den")
                        av, den = avs[t], dens[t]
                        # weight V rows in-place (DVE, fully packed bf16)
                        nc.vector.tensor_tensor(
                            out=kvg[:, :, DIM:],
                            in0=kvg[:, :, DIM:],
                            in1=eex[:].rearrange("p k h d -> p k (h d)"),
                            op=mybir.AluOpType.mult)
                        # pairwise tree-reduce over the 16 neighbors
                        with nc.allow_low_precision(reason="bf16 flash accum, f32 final"):
                            nc.vector.tensor_tensor(
                                out=kvg[:, :8, DIM:], in0=kvg[:, :8, DIM:],
                                in1=kvg[:, 8:, DIM:], op=mybir.AluOpType.add)
                            nc.gpsimd.tensor_tensor(
                                out=kvg[:, :4, DIM:], in0=kvg[:, :4, DIM:],
                                in1=kvg[:, 4:8, DIM:], op=mybir.AluOpType.add)
                            nc.gpsimd.tensor_tensor(
                                out=kvg[:, :2, DIM:], in0=kvg[:, :2, DIM:],
                                in1=kvg[:, 2:4, DIM:], op=mybir.AluOpType.add)
                        dpart = spool.tile([128, HEADS], f32, tag="dpart", name="dpart")
                        nc.vector.tensor_reduce(
                            out=(den[:] if h2 == 0 else dpart[:]),
                            in_=eex[:, :, :, 0].rearrange("p k h -> p h k"),
                            axis=mybir.AxisListType.X,
                            op=mybir.AluOpType.add)
                        if h2 == 0:
                            nc.gpsimd.tensor_tensor(
                                out=av[:], in0=kvg[:, 0, DIM:], in1=kvg[:, 1, DIM:],
                                op=mybir.AluOpType.add)
                        else:
                            rsum = rpool.tile([128, DIM], f32, tag="rsum", name="rsum")
                            nc.gpsimd.tensor_tensor(
                                out=rsum[:], in0=kvg[:, 0, DIM:], in1=kvg[:, 1, DIM:],
                                op=mybir.AluOpType.add)
                            nc.gpsimd.tensor_tensor(
                                out=av[:], in0=av[:], in1=rsum[:],
                                op=mybir.AluOpType.add)
                            nc.vector.tensor_tensor(
                                out=den[:], in0=den[:], in1=dpart[:],
                                op=mybir.AluOpType.add)

                    aos = {}

                    def tail1(t):
                        av, den = avs.pop(t), dens.pop(t)
                        rden = apool.tile([128, HEADS], f32, tag="rden", name="rden")
                        nc.vector.reciprocal(out=rden[:], in_=den[:])
                        ao = apool.tile([128, DIM], bf16, tag="ao", name="ao")
                        aos[t] = ao
                        nc.vector.tensor_tensor(
                            out=ao[:].rearrange("p (h d) -> p h d", h=HEADS),
                            in0=av[:].rearrange("p (h d) -> p h d", h=HEADS),
                            in1=rden[:].rearrange(
                                "p (h o) -> p h o", o=1).to_broadcast([128, HEADS, HD]),
                            op=mybir.AluOpType.mult)

                    def tail2(t):
                        ao = aos.pop(t)
                        pst = psT.tile([128, DIM], bf16, tag="tp", name="pst")
                        for c in range(4):
                            nc.tensor.transpose(
                                out=pst[:, c * 128:(c + 1) * 128],
                                in_=ao[:, c * 128:(c + 1) * 128],
                                identity=ident[:])
                        aT = apool.tile([128, DIM], bf16, tag="aT", name="aT")
                        nc.scalar.activation(
                            out=aT[:], in_=pst[:],
                            func=mybir.ActivationFunctionType.Copy)
                        pso = psO.tile([128, DIN], f32, tag="mo", name="pso")
                        for c in range(4):
                            nc.tensor.matmul(
                                out=pso[:], lhsT=aT[:, c * 128:(c + 1) * 128],
                                rhs=wo_sb[c][:], start=(c == 0), stop=(c == 3))
                        ot = apool.tile([128, DIN], f32, tag="ot", name="ot")
                        nc.vector.tensor_tensor(
                            out=ot[:], in0=pso[:], in1=bo_sb[:],
                            op=mybir.AluOpType.add)
                        nc.sync.dma_start(
                            out=out_h[t * 128:(t + 1) * 128, :], in_=ot[:])

                    for j in range(NJ + 2):
                        if j < NJ:
                            score_phase(j)
                        if 1 <= j <= NJ:
                            av_phase(j - 1)
                        if j < NJ:
                            score_back(j)
                        if 1 <= j <= NJ and (j - 1) % NH == NH - 1:
                            tail1((j - 1) // NH)
                        if 2 <= j and (j - 2) % NH == NH - 1 and j - 2 >= 0:
                            tail2((j - 2) // NH)
    nc.compile()
    return nc


def _get_nc():
    if "nc" not in _CACHE:
        _CACHE["nc"] = _build()
    return _CACHE["nc"]


def kernel(**inputs) -> np.ndarray:
    from concourse.bass_utils import run_bass_kernel_spmd
    from ml_dtypes import bfloat16

    x = np.asarray(inputs["x"], dtype=np.float32)
    ctx = np.asarray(inputs["context"], dtype=np.float32)
    idx = np.asarray(inputs["index_pairs"]).astype(np.int64)
    scale = 1.0 / np.sqrt(HD)
    wq = (np.asarray(inputs["Wq"], dtype=np.float32) * scale).astype(bfloat16)
    bq = np.tile((np.asarray(inputs["bq"], dtype=np.float32) * scale).reshape(1, DIM),
                 (128, 1)).astype(np.float32)
    wk = np.asarray(inputs["Wk"], dtype=np.float32).astype(bfloat16)
    wv = np.asarray(inputs["Wv"], dtype=np.float32).astype(bfloat16)
    wo = np.asarray(inputs["Wout"], dtype=np.float32).astype(bfloat16)
    bo = np.tile(np.asarray(inputs["bout"], dtype=np.float32).reshape(1, DIN),
                 (128, 1)).astype(np.float32)

    nc = _get_nc()
    in_maps = []
    for c in range(8):
        b, half = c // 2, c % 2
        xT_c = np.ascontiguousarray(
            x[b, half * N_LOC:(half + 1) * N_LOC, :].T).astype(bfloat16)
        cT_c = np.ascontiguousarray(ctx[b].T).astype(bfloat16)
        idx_c = idx[b, half * N_LOC:(half + 1) * N_LOC, :].astype(np.int16)  # [1024, 32]
        # dma_gather channel layout: per (tile, half, quarter) block of 64
        # cols, item i = kk*128 + q lives at [i % 16, block*64 + i // 16],
        # replicated to all 8 GPSIMD-core partition groups.
        blocks = []
        for t in range(NT):
            for h2 in range(NH):
                for g in range(2):
                    k0 = h2 * KH + g * (KH // 2)
                    sub = idx_c[t * 128:(t + 1) * 128, k0:k0 + KH // 2]  # [128 q, 8 kk]
                    items = sub.T.reshape(-1)  # items[kk*128 + q]
                    blocks.append(items.reshape(64, 16).T)  # [16, 64]
        idx_w = np.tile(np.concatenate(blocks, axis=1), (8, 1))
        in_maps.append({
            "xT": xT_c, "cT": cT_c, "idx": idx_w,
            "wq": wq, "wk": wk, "wv": wv, "wo": wo, "bq": bq, "bo": bo,
        })
    res = run_bass_kernel_spmd(nc, in_maps, core_ids=list(range(8)))
    out = np.empty((B, N, DIN), dtype=np.float32)
    for c in range(8):
        b, half = c // 2, c % 2
        out[b, half * N_LOC:(half + 1) * N_LOC, :] = res.results[c]["out"]
    return out
